# revision 18
# baseline (speedup 1.0000x reference)
"""Entropic OT loss (Sinkhorn) kernel for Trainium2, 8 NeuronCores.

Algorithm: the reference's stabilized log-domain Sinkhorn equals standard
u/v-scaling Sinkhorn on K = exp(-M/reg).  Rescaling invariance: with
u' = u/sqrt(a), v' = v/sqrt(a) the iteration becomes u' = 1/(K v'),
v' = 1/(K^T u'), and loss = a * u'^T (K o M) v' — so the per-halfstep
device chain is a single DVE reciprocal (bf16 out), no scale op.

Each of S=24 problems: K is a Gaussian kernel of 1024 points in R^3,
built on device via rank-15 (bf16 hi/lo split) matmuls + fused exp with
per-partition bias.  The exp activations also emit accum_out row sums,
which give u(1) = 1/(K*ones) for free (v0 = ones).  19 v-matvecs and
19 u-matvecs run as PE matvec pairs (bf16 weights, f32 PSUM).  The last
v-update is folded into the final batched free=5 matmul:
  (K o M)^T u = nrj o (K^T u) + K^T(nri o u) - 2 sum_c rj_c o K^T(ri_c o u)
and since v = 1/(K^T u), the nrj term contributes sum(nrj) exactly —
computed host-side.  Device returns per-partition partial sums.

Sharding: 24 problems -> 8 cores x 3.
"""

import numpy as np
import ml_dtypes

from concourse import bass, mybir
from concourse.tile import TileContext
from concourse.bass_utils import run_bass_kernel_spmd

BF16 = ml_dtypes.bfloat16

B, N, C, H, W = 8, 5, 3, 32, 32
D = H * W              # 1024
S = 24                 # B * K_PAIRS
NITER = 20
NCORES = 8
PPC = S // NCORES      # 3 problems per core
NB = D // 128          # 8
A_MARG = 1.0 / D

FP32 = mybir.dt.float32
BF16_DT = mybir.dt.bfloat16

# constF column layout
C_BIASK = 0            # 8 cols per problem (-2*nri, d-layout)
C_BIASKT = 24          # 8 cols per problem (-2*nrj, e-layout)
C_FINRI = 48           # 32 cols per problem: [nri, ri_0, ri_1, ri_2] d-layout
C_FINRJ = 144          # 24 cols per problem: [rj_0, rj_1, rj_2] e-layout
C_WSCL = 216           # 1 col per problem (w*a/S broadcast on partitions)
C_TOT = 219


def _split_hilo(x):
    hi = x.astype(BF16)
    lo = (x - hi.astype(np.float32)).astype(BF16)
    return hi, lo


def _split3(x):
    """f32 -> 3 bf16 terms summing to x to ~2e-8 rel."""
    h1 = x.astype(BF16)
    r = x - h1.astype(np.float32)
    h2 = r.astype(BF16)
    h3 = (r - h2.astype(np.float32)).astype(BF16)
    return (h1.astype(np.float32), h2.astype(np.float32), h3.astype(np.float32))


def _dlayout(x):
    """[1024] -> [128, 8] with d = db*128 + dp at [dp, db]."""
    return np.ascontiguousarray(x.reshape(NB, 128).T)


def build_program():
    nc = bass.Bass(target_bir_lowering=False, num_swdge_queues=4)

    # [15 ch, p, side(K=0/KT=1), stat(0)/mov(1), 1024]
    opsBF = nc.dram_tensor("opsBF", [15, PPC, 2, 2, D], BF16_DT,
                           kind="ExternalInput")
    constF = nc.dram_tensor("constF", [128, C_TOT], FP32, kind="ExternalInput")
    out_par = nc.dram_tensor("partials", [128, PPC], FP32, kind="ExternalOutput")

    with TileContext(nc) as tc:
        with tc.tile_pool(name="const", bufs=1) as cpool, \
             tc.tile_pool(name="kmat", bufs=1) as kpool, \
             tc.tile_pool(name="work", bufs=1) as wpool, \
             tc.tile_pool(name="psA", bufs=5, space="PSUM") as psA, \
             tc.tile_pool(name="psI", bufs=1, space="PSUM") as psI:

            cf_sb = cpool.tile([128, C_TOT], FP32, tag="cf")
            nc.sync.dma_start(out=cf_sb[:, :], in_=constF[:, :])

            ops_sb = [cpool.tile([15, 2, 2, D], BF16_DT, tag=f"ops{p}",
                                 name=f"ops{p}")
                      for p in range(PPC)]
            # three concurrent DMA queues: sync (HWDGE), scalar (HWDGE),
            # gpsimd (SWDGE); earliest-needed pieces first per queue.
            nc.sync.dma_start(out=ops_sb[0][:, 0], in_=opsBF[:, 0, 0])
            nc.scalar.dma_start(out=ops_sb[0][:, 1], in_=opsBF[:, 0, 1])
            nc.sync.dma_start(out=ops_sb[1][:, 0], in_=opsBF[:, 1, 0])
            nc.scalar.dma_start(out=ops_sb[1][:, 1], in_=opsBF[:, 1, 1])
            nc.gpsimd.dma_start(out=ops_sb[2][:, 0], in_=opsBF[:, 2, 0])
            nc.gpsimd.dma_start(out=ops_sb[2][:, 1], in_=opsBF[:, 2, 1])

            def finri_ap(p, c):        # c=0 -> nri, c=1..3 -> ri_{c-1}
                o = C_FINRI + 32 * p + 8 * c
                return cf_sb[:, o:o + 8]

            def finrj_ap(p, c):        # rj_c, e-layout
                o = C_FINRJ + 24 * p + 8 * c
                return cf_sb[:, o:o + 8]

            # ---- build KT (side 1) then K (side 0) per problem, bf16 ----
            # PE matmul -> PSUM -> DVE copy -> deep SBUF ring -> ACT exp.
            # The ring decouples ACT pacing from the PE's 64-MM iteration
            # bursts so the exp chain runs back-to-back.  Chunk pairs
            # (ob, h=0/1) share the per-partition bias, so one [128,1024]
            # exp covers both.
            K_sb = [kpool.tile([128, NB * D], BF16_DT, tag=f"K{p}", name=f"K{p}")
                    for p in range(PPC)]
            KT_sb = [kpool.tile([128, NB * D], BF16_DT, tag=f"KT{p}", name=f"KT{p}")
                     for p in range(PPC)]
            RING = 8
            stage = cpool.tile([128, RING, 2, 512], FP32, tag="stage")
            pair_ctr = 0

            for p in range(PPC):
                for side in (1, 0):
                    dst = K_sb[p] if side == 0 else KT_sb[p]
                    bias_col = (C_BIASK if side == 0 else C_BIASKT) + NB * p
                    for ob in range(NB):
                        slot = pair_ctr % RING
                        pair_ctr += 1
                        for h in range(2):
                            ps = psA.tile([128, 512], FP32, tag="psA")
                            nc.tensor.matmul(
                                out=ps[:, :],
                                lhsT=ops_sb[p][:, side, 0, ob * 128:(ob + 1) * 128],
                                rhs=ops_sb[p][:, side, 1, h * 512:(h + 1) * 512],
                                start=True, stop=True,
                            )
                            with tc.high_priority(offset=500_000):
                                nc.vector.tensor_copy(stage[:, slot, h, :],
                                                      ps[:, :])
                        nc.scalar.activation(
                            out=dst[:, ob * D: (ob + 1) * D],
                            in_=stage[:, slot, :, :],
                            func=mybir.ActivationFunctionType.Exp,
                            bias=cf_sb[:, bias_col + ob: bias_col + ob + 1],
                            scale=1.0,
                        )

            # ---- iterations (rescaled: u = 1/(Kv), v = 1/(K^T u)) ----
            ub = [None] * PPC
            vb = [None] * PPC
            uf = [None] * PPC

            with nc.allow_low_precision(reason="bf16 sinkhorn scaling vectors"):
                # per-problem gate: ones_p = (p's last built K chunk)*0 + 1.
                # Problem p's whole iteration chain hangs off ones_p, so its
                # matmuls cannot be scheduled before p's own build is done,
                # but they do fill the PE while later problems still build.
                onesl = []
                for p in range(PPC):
                    onesp = wpool.tile([128, 1], BF16_DT, tag=f"ones{p}")
                    with tc.high_priority(offset=1_000_000):
                        nc.vector.tensor_scalar(
                            out=onesp[:, :], in0=K_sb[p][:, NB * D - 1: NB * D],
                            scalar1=0.0, scalar2=1.0,
                            op0=mybir.AluOpType.mult, op1=mybir.AluOpType.add)
                    onesl.append(onesp)
                # u(1) = 1/(K @ ones): ones-matvec on the PE (KT weights)
                for p in range(PPC):
                    ps = psI.tile([128, NB], FP32, tag=f"ps{p}")
                    for db in range(NB):
                        for eb in range(NB):
                            nc.tensor.matmul(
                                out=ps[:, db:db + 1],
                                lhsT=KT_sb[p][:, eb * D + db * 128:
                                              eb * D + (db + 1) * 128],
                                rhs=onesl[p][:, 0:1],
                                start=(eb == 0), stop=(eb == NB - 1),
                            )
                    u1 = wpool.tile([128, NB], BF16_DT, tag=f"ub{p}")
                    with tc.high_priority(offset=1_000_000):
                        nc.vector.reciprocal(out=u1[:, :], in_=ps[:, :])
                    ub[p] = u1

                for t in range(1, NITER):
                    last = (t == NITER - 1)
                    for p in range(PPC):   # v(t) = 1/(K^T u(t)); lhsT = K blocks
                        ps = psI.tile([128, NB], FP32, tag=f"ps{p}")
                        # last-built problem gets elevated priority so its
                        # late start interleaves into the others' stalls
                        with tc.high_priority(offset=500_000 if p == PPC - 1
                                              else 0):
                            for eb in range(NB):
                                for db in range(NB):
                                    nc.tensor.matmul(
                                        out=ps[:, eb:eb + 1],
                                        lhsT=K_sb[p][:, db * D + eb * 128:
                                                     db * D + (eb + 1) * 128],
                                        rhs=ub[p][:, db:db + 1],
                                        start=(db == 0), stop=(db == NB - 1),
                                    )
                        v = wpool.tile([128, NB], BF16_DT, tag=f"vb{p}")
                        with tc.high_priority(offset=1_000_000):
                            nc.vector.reciprocal(out=v[:, :], in_=ps[:, :])
                        vb[p] = v
                    for p in range(PPC):   # u(t+1) = 1/(K v(t)); lhsT = KT blocks
                        ps = psI.tile([128, NB], FP32, tag=f"ps{p}")
                        with tc.high_priority(offset=500_000 if p == PPC - 1
                                              else 0):
                            for db in range(NB):
                                for eb in range(NB):
                                    nc.tensor.matmul(
                                        out=ps[:, db:db + 1],
                                        lhsT=KT_sb[p][:, eb * D + db * 128:
                                                      eb * D + (db + 1) * 128],
                                        rhs=vb[p][:, eb:eb + 1],
                                        start=(eb == 0), stop=(eb == NB - 1),
                                    )
                        if last:
                            u20 = wpool.tile([128, NB], FP32, tag=f"uf{p}")
                            with tc.high_priority(offset=1_000_000):
                                nc.vector.reciprocal(out=u20[:, :], in_=ps[:, :])
                            uf[p] = u20
                        else:
                            u = wpool.tile([128, NB], BF16_DT, tag=f"ub{p}")
                            with tc.high_priority(offset=1_000_000):
                                nc.vector.reciprocal(out=u[:, :], in_=ps[:, :])
                            ub[p] = u

                # ---- final: psF3 = K^T [u, nri*u, ri_c*u]; v(20) = 1/col0 ----
                par_sb = wpool.tile([128, PPC], FP32, tag="par")
                for p in range(PPC):
                    rhs5 = wpool.tile([128, NB, 5], BF16_DT, tag=f"rhs5{p}")
                    with tc.high_priority(offset=1_000_000):
                        nc.vector.tensor_copy(rhs5[:, :, 0], uf[p][:, :])
                        for c in range(4):
                            nc.vector.tensor_mul(rhs5[:, :, 1 + c],
                                                 finri_ap(p, c), uf[p][:, :])
                    psF = psI.tile([128, NB, 5], FP32, tag=f"ps{p}")
                    for eb in range(NB):
                        for db in range(NB):
                            nc.tensor.matmul(
                                out=psF[:, eb, :],
                                lhsT=K_sb[p][:, db * D + eb * 128:
                                             db * D + (eb + 1) * 128],
                                rhs=rhs5[:, db, :],
                                start=(db == 0), stop=(db == NB - 1),
                            )
                    inv = wpool.tile([128, NB], FP32, tag=f"inv{p}")
                    nc.vector.reciprocal(out=inv[:, :], in_=psF[:, :, 0])
                    tt = wpool.tile([128, NB], FP32, tag=f"t{p}")
                    qq = wpool.tile([128, NB], FP32, tag=f"q{p}")
                    nc.vector.tensor_mul(qq[:, :], finrj_ap(p, 0), psF[:, :, 2])
                    nc.vector.scalar_tensor_tensor(
                        out=tt[:, :], in0=qq[:, :], scalar=-2.0,
                        in1=psF[:, :, 1],
                        op0=mybir.AluOpType.mult, op1=mybir.AluOpType.add)
                    for c in range(1, 3):
                        nc.vector.tensor_mul(qq[:, :], finrj_ap(p, c),
                                             psF[:, :, 2 + c])
                        nc.vector.scalar_tensor_tensor(
                            out=tt[:, :], in0=qq[:, :], scalar=-2.0,
                            in1=tt[:, :],
                            op0=mybir.AluOpType.mult, op1=mybir.AluOpType.add)
                    dump = wpool.tile([128, NB], FP32, tag=f"dump{p}")
                    nc.vector.scalar_tensor_tensor(
                        out=dump[:, :], in0=tt[:, :],
                        scalar=cf_sb[:, C_WSCL + p: C_WSCL + p + 1],
                        in1=inv[:, :],
                        op0=mybir.AluOpType.mult, op1=mybir.AluOpType.mult,
                        accum_out=par_sb[:, p:p + 1])

            nc.gpsimd.dma_start(out=out_par[:, :], in_=par_sb[:, :])

    return nc



def _strip_redundant_incs(nc):
    """Tick-semaphore increments cost ~26ns each serialized on the engine's
    event path; every MM carries one but only ~5% of counts are ever waited
    on.  Strip increments whose cumulative count no wait references, and
    remap the remaining wait thresholds.  Only touches semaphores whose
    increments all come from one engine's non-DMA instructions (in-order
    completion) with unit sem-inc updates and whose waits are all
    sem-ge-imm."""
    import json as _json
    bir = _json.loads(nc.to_json_bytes())

    blocks = [blk for fn in bir["functions"] for blk in fn["blocks"]]
    # gather per-sem info across all blocks
    upd = {}    # sem id -> list of (block idx, inst idx, upd entry idx)
    upd_ok = {}  # sem id -> bool (eligible)
    waits = {}  # sem id -> list of wait dicts
    for bi, blk in enumerate(blocks):
        for ii, inst in enumerate(blk["instructions"]):
            si = inst.get("sync_info") or {}
            for ui, u in enumerate(si.get("on_update") or []):
                s = u.get("id")
                upd.setdefault(s, []).append((bi, ii, ui))
                ok = (u.get("sync_type") == "semaphore"
                      and u.get("update_mode") == "sem-inc"
                      and u.get("update_value") == 1
                      and inst["opcode"] not in ("DMACopy", "Call", "ISA")
                      and inst["engine"] not in ("Unassigned",))
                eng = inst["engine"]
                prev = upd_ok.get(s)
                if prev is None:
                    upd_ok[s] = ok and (eng,)
                elif prev and prev == (eng,) and ok:
                    pass
                else:
                    upd_ok[s] = False
            for w in (si.get("on_wait") or []):
                s = w.get("id")
                waits.setdefault(s, []).append(w)

    for s, incs in upd.items():
        if not upd_ok.get(s):
            continue
        ws = waits.get(s, [])
        if any(w.get("wait_mode") != "sem-ge-imm" for w in ws):
            continue
        needed = sorted({w["wait_value"] for w in ws if w["wait_value"] > 0})
        if not needed or needed[-1] > len(incs):
            continue
        needed_set = set(needed)
        rank = {k: r + 1 for r, k in enumerate(needed)}
        # strip unneeded increments (1-indexed position in inc order)
        for pos, (bi, ii, ui) in enumerate(incs, start=1):
            if pos not in needed_set:
                si = blocks[bi]["instructions"][ii]["sync_info"]
                si["on_update"] = [u for u in si["on_update"]
                                   if u.get("id") != s]
        # remap wait thresholds
        for w in ws:
            if w["wait_value"] > 0:
                w["wait_value"] = rank[w["wait_value"]]

    fixed = _json.dumps(bir).encode()
    nc.to_json_bytes = lambda: fixed
    return nc


def _split_multi_waits(nc):
    """This walrus build accepts at most one sync wait per instruction.
    Tile emits up to two. Split surplus waits onto injected EventSemaphore
    nops placed immediately before the instruction in its engine stream."""
    import json as _json
    bir = _json.loads(nc.to_json_bytes())
    ctr = 0
    for fn in bir["functions"]:
        for blk in fn["blocks"]:
            new_insts = []
            for inst in blk["instructions"]:
                si = inst.get("sync_info")
                ow = (si or {}).get("on_wait") or []
                if len(ow) > 1:
                    for w in ow[:-1]:
                        ctr += 1
                        new_insts.append({
                            "engine": inst["engine"], "ins": [], "outs": [],
                            "name": f"waitsplit-{ctr}",
                            "opcode": "EventSemaphore",
                            "sync_info": {"on_update": [], "on_wait": [w]},
                        })
                    si["on_wait"] = [ow[-1]]
                new_insts.append(inst)
            blk["instructions"] = new_insts
    fixed = _json.dumps(bir).encode()
    nc.to_json_bytes = lambda: fixed
    return nc


_NC_CACHE = None


def _get_program():
    global _NC_CACHE
    if _NC_CACHE is None:
        _NC_CACHE = _split_multi_waits(_strip_redundant_incs(build_program()))
    return _NC_CACHE


def _prep_inputs(burst, gt_img, indices):
    burst = np.asarray(burst, np.float32)
    gt = np.asarray(gt_img, np.float32)
    idx = np.asarray(indices)
    diffs = (gt[:, None] - burst).reshape(B, N, C, D).transpose(0, 1, 3, 2)
    ri = diffs[idx[:, 0], idx[:, 2]]  # [S,D,C]
    rj = diffs[idx[:, 1], idx[:, 3]]
    nri = np.sum(ri * ri, -1)
    nrj = np.sum(rj * rj, -1)
    w = 0.5 * (ri.mean(axis=(1, 2)) + rj.mean(axis=(1, 2)))

    in_maps = []
    host_consts = []
    for core in range(NCORES):
        ops = np.zeros((15, PPC, 2, 2, D), BF16)
        cf = np.zeros((128, C_TOT), np.float32)
        hc = 0.0
        for p in range(PPC):
            s = core * PPC + p
            ri_hi, ri_lo = _split_hilo(ri[s])
            rj_hi, rj_lo = _split_hilo(rj[s])
            ones = np.ones(D, BF16)

            def stat_side(x_hi, x_lo):
                return np.concatenate(
                    [x_hi.T, x_hi.T, x_lo.T, x_lo.T,
                     ones[None], ones[None], ones[None]], axis=0)

            def mov_side(y_hi, y_lo, nrm):
                n1, n2, n3 = nrm
                return np.concatenate(
                    [4 * y_hi.T.astype(np.float32), 4 * y_lo.T.astype(np.float32),
                     4 * y_hi.T.astype(np.float32), 4 * y_lo.T.astype(np.float32),
                     n1[None], n2[None], n3[None]], axis=0).astype(BF16)

            nrj3 = _split3(-2.0 * nrj[s])
            nri3 = _split3(-2.0 * nri[s])
            ops[:, p, 0, 0] = stat_side(ri_hi, ri_lo)          # K stat (d cols)
            ops[:, p, 0, 1] = mov_side(rj_hi, rj_lo, nrj3)     # K mov (e cols)
            ops[:, p, 1, 0] = stat_side(rj_hi, rj_lo)          # KT stat (e cols)
            ops[:, p, 1, 1] = mov_side(ri_hi, ri_lo, nri3)     # KT mov (d cols)

            cf[:, C_BIASK + NB * p: C_BIASK + NB * (p + 1)] = \
                _dlayout(-2.0 * nri[s])
            cf[:, C_BIASKT + NB * p: C_BIASKT + NB * (p + 1)] = \
                _dlayout(-2.0 * nrj[s])
            cf[:, C_FINRI + 32 * p: C_FINRI + 32 * p + 8] = _dlayout(nri[s])
            for c in range(C):
                cf[:, C_FINRI + 32 * p + 8 * (1 + c):
                   C_FINRI + 32 * p + 8 * (2 + c)] = \
                    _dlayout(np.ascontiguousarray(ri[s][:, c]))
                cf[:, C_FINRJ + 24 * p + 8 * c:
                   C_FINRJ + 24 * p + 8 * (c + 1)] = \
                    _dlayout(np.ascontiguousarray(rj[s][:, c]))
            wscl = np.float32(w[s]) * np.float32(A_MARG) / np.float32(S)
            cf[:, C_WSCL + p] = wscl
            hc += float(wscl) * float(np.sum(nrj[s], dtype=np.float64))
        in_maps.append({"opsBF": ops, "constF": cf})
        host_consts.append(hc)
    return in_maps, host_consts


def kernel(burst, gt_img, indices):
    nc = _get_program()
    in_maps, host_consts = _prep_inputs(burst, gt_img, indices)
    res = run_bass_kernel_spmd(nc, in_maps, list(range(NCORES)))
    total = np.float64(0.0)
    for core in range(NCORES):
        total += np.float64(res.results[core]["partials"]
                            .astype(np.float64).sum())
        total += host_consts[core]
    return np.float32(total)
